# revision 2
# baseline (speedup 1.0000x reference)
"""Trainium2 Bass kernel for a BasicTransformerBlock (B=2, S=2048, H=768, FF=3072, NH=12).

Sharding: core c handles batch b=c//4, sequence quarter q=c%4 (512 tokens).
Each core redundantly computes LN1 + K/V projections for its batch's full
2048 tokens (no collectives); Q/attention/Wo/FFN only for its own 512 tokens.

v2 over the bf16 baseline:
- fp8(e4m3, x64 pre-scale) DoubleRow matmuls for QKV projections, Wo and the
  FFN W2 GEMM (2 contraction chunks per instruction -> ~2x the bf16 rate).
  W1 stays bf16 (the z=nx@W1 path dominates the quantization error budget;
  this config lands ~1.3e-2 rel L2 vs the 2e-2 gate).
- Wo consumes per-head-pair ctx tiles [64, 2, T] via DoubleRow, which sums the
  two heads' contributions and removes the partition-shift DMAs the old
  attention tail needed.  Softmax denominators: ACT copy shifts the psum row
  to partition 0 (custom DVE ops require offset-0 operands), DVE fast-approx
  reciprocal, gpsimd partition_broadcast, and a x16 pre-scale on the fp8 ctx
  store to stay out of e4m3's denormal range (undone in the Wo evacuation).
- LN1 stats (ones-row matmuls on x and ACT-squared x) for all 4 token tiles
  are emitted up front; the LN apply is 2 fused whole-tile DVE ops using
  stride-0 chunk-broadcast APs of alpha/beta.
- Phase-1 ACT functions kept to one table set (Identity/Square/Sqrt/Copy) to
  avoid mid-phase ACT table reloads; exp is the only attention ACT function.
- W2/Wo/W1-tiles prefetched on the scalar DMA ring in need-order; W1 streams
  per-mh on the sync ring during the FFN.

Host-side folds (f32): Wq_eff = diag(ln1_w) Wq, bq_eff = ln1_b@Wq + bq (same
k/v); bo_eff = (ln1_b@Wv + bv)@Wo + bo; W1_eff = diag(ln2_w) W1,
b1_eff = ln2_b@W1 + b1.  fp8 weights are scaled by 64 before the e4m3 cast
(undone at PSUM evacuation) so weight magnitudes sit in e4m3's normal range.
"""

import os
import numpy as np
import ml_dtypes

DEBUG_TAPS = bool(int(os.environ.get("KDBG", "0")))

import concourse.bass as bass
import concourse.tile as tile
from concourse import bacc, mybir
from concourse.bass import ts, ds
from concourse.alu_op_type import AluOpType
from concourse.bass_utils import run_bass_kernel_spmd

F32 = mybir.dt.float32
BF16 = mybir.dt.bfloat16
FP8 = mybir.dt.float8e4
AF = mybir.ActivationFunctionType
DR = mybir.MatmulPerfMode.DoubleRow

H = 768
FF = 3072
NH = 12
DH = 64
B = 2
S = 2048
P = 128
NCORES = 8
TQ = 512          # own tokens per core
NTT = S // TQ     # 4 token tiles per batch
FC = H // P       # 6 feature chunks
FCP = FC // 2     # 3 feature chunk pairs
FFC = FF // P     # 24 hidden chunks
FFCP = FFC // 2   # 12 hidden chunk pairs
TKC = S // P      # 16 key token chunks
HPAIRS = NH // 2  # 6 head pairs
TH = TQ // 2      # FFN token half
EPS = 1e-6
WS = 64.0         # fp8 weight pre-scale
RWS = 1.0 / WS


def build():
    nc = bacc.Bacc("TRN2", target_bir_lowering=False, debug=False,
                   num_devices=NCORES)

    latq_d = nc.dram_tensor("latTq", [H, TQ], F32, kind="ExternalInput")
    latbf_d = nc.dram_tensor("latTbf", [H, S], BF16, kind="ExternalInput")
    wq_d = nc.dram_tensor("wq", [P, FC, H], FP8, kind="ExternalInput")
    wk_d = nc.dram_tensor("wk", [P, FC, H], FP8, kind="ExternalInput")
    wv_d = nc.dram_tensor("wv", [P, FC, H], FP8, kind="ExternalInput")
    wo_d = nc.dram_tensor("wo", [DH, HPAIRS, 2, H], FP8, kind="ExternalInput")
    w1_d = nc.dram_tensor("w1", [FFC, P, FC, P], BF16, kind="ExternalInput")
    w2_d = nc.dram_tensor("w2", [P, FFCP, 2, H], FP8, kind="ExternalInput")
    bq_d = nc.dram_tensor("bq", [P, FC], F32, kind="ExternalInput")
    bk_d = nc.dram_tensor("bk", [P, FC], F32, kind="ExternalInput")
    bo_d = nc.dram_tensor("bo", [P, FC], F32, kind="ExternalInput")
    b1_d = nc.dram_tensor("b1", [P, FFC], F32, kind="ExternalInput")
    b2_d = nc.dram_tensor("b2", [P, FC], F32, kind="ExternalInput")
    out_d = nc.dram_tensor("outT", [H, TQ], F32, kind="ExternalOutput")
    if DEBUG_TAPS:
        dbg_lat2_d = nc.dram_tensor("dbg_lat2", [P, FC, TQ], F32,
                                    kind="ExternalOutput")
        dbg_nx2_d = nc.dram_tensor("dbg_nx2", [P, FC, TQ], BF16,
                                   kind="ExternalOutput")
        dbg_ctx_d = nc.dram_tensor("dbg_ctx", [DH, 2, TQ], FP8,
                                   kind="ExternalOutput")
        dbg_k_d = nc.dram_tensor("dbg_k", [P, FC, TQ], BF16,
                                 kind="ExternalOutput")

    latq_ap = latq_d.ap().rearrange("(c p) t -> p c t", p=P)
    latbf_ap = latbf_d.ap().rearrange("(c p) t -> p c t", p=P)
    out_ap = out_d.ap().rearrange("(c p) t -> p c t", p=P)

    with tile.TileContext(nc) as tc:
        with (
            tc.tile_pool(name="consts", bufs=1) as consts,
            tc.tile_pool(name="persist", bufs=1) as persist,
        ):
            # constants (vector ring for the small bias DMAs)
            ones_col_bf = consts.tile([P, 1], BF16)
            nc.vector.memset(ones_col_bf[:], 1.0)
            eps_tile = consts.tile([1, 1], F32)
            nc.vector.memset(eps_tile[:], EPS)
            zero_col = consts.tile([P, 1], F32)
            nc.vector.memset(zero_col[:], 0.0)
            bq_sb = consts.tile([P, FC], F32)
            nc.gpsimd.dma_start(bq_sb[:], bq_d.ap())
            bk_sb = consts.tile([P, FC], F32)
            nc.gpsimd.dma_start(bk_sb[:], bk_d.ap())
            bo_sb = consts.tile([P, FC], F32)
            nc.gpsimd.dma_start(bo_sb[:], bo_d.ap())
            b1_sb = consts.tile([P, FFC], F32)
            nc.gpsimd.dma_start(b1_sb[:], b1_d.ap())
            b2_sb = consts.tile([P, FC], F32)
            nc.gpsimd.dma_start(b2_sb[:], b2_d.ap())

            # persistent activations
            kT = [persist.tile([P, FC, TQ], BF16, tag=f"kT{t}",
                               name=f"kT{t}")
                  for t in range(NTT)]
            v_sb = persist.tile([P, TKC, NH, DH + 1], BF16)
            nc.vector.memset(v_sb[:, :, :, DH:DH + 1], 1.0)
            qT = persist.tile([P, FC, TQ], BF16)
            ctxP = [persist.tile([DH, 2, TQ], FP8, tag=f"ctxP{hp}",
                                 name=f"ctxP{hp}")
                    for hp in range(HPAIRS)]
            resid1 = persist.tile([P, FC, TQ], F32, tag="bigf32")
            lat2T = persist.tile([P, FC, TQ], F32, tag="lat2")
            nx2 = persist.tile([P, FC, TQ], BF16, tag="nx2")

            wo_sb = persist.tile([DH, HPAIRS, 2, H], FP8, tag="wo")

            def ln_tail(T, ps_sum, ps_sq, small_pool, ab_pool):
                """sum/sqsum rows -> broadcast alpha/beta [P,T] bf16 tiles."""
                mu = small_pool.tile([1, T], F32, tag="lnsmall")
                nc.scalar.mul(mu[:], ps_sum, 1.0 / H)
                mu2 = small_pool.tile([1, T], F32, tag="lnsmall")
                nc.vector.tensor_mul(mu2[:], mu[:], mu[:])
                msq = small_pool.tile([1, T], F32, tag="lnsmall")
                nc.scalar.mul(msq[:], ps_sq, 1.0 / H)
                var = small_pool.tile([1, T], F32, tag="lnsmall")
                nc.vector.tensor_sub(var[:], msq[:], mu2[:])
                sd = small_pool.tile([1, T], F32, tag="lnsmall")
                nc.scalar.activation(sd[:], var[:], AF.Sqrt, bias=eps_tile[:])
                rsig = small_pool.tile([1, T], F32, tag="lnsmall")
                nc.vector.reciprocal_approx_fast(rsig[:], sd[:])
                rsig_bf = small_pool.tile([1, T], BF16, tag="lnsmallbf")
                nc.scalar.copy(rsig_bf[:], rsig[:])
                beta_bf = small_pool.tile([1, T], BF16, tag="lnsmallbf")
                nc.vector.scalar_tensor_tensor(beta_bf[:], mu[:], -1.0, rsig[:],
                                               AluOpType.mult, AluOpType.mult)
                ab = ab_pool.tile([P, T], BF16, tag="ab")
                nc.gpsimd.partition_broadcast(ab[:], rsig_bf[0:1, :])
                bb = ab_pool.tile([P, T], BF16, tag="bb")
                nc.gpsimd.partition_broadcast(bb[:], beta_bf[0:1, :])
                return ab, bb

            # ---------------- Phase 1: LN1 + K/V/Q projections ----------------
            with (
                tc.tile_pool(name="wproj", bufs=1) as wproj,
                tc.tile_pool(name="latp", bufs=4) as latp,
                tc.tile_pool(name="sqp", bufs=2) as sqp,
                tc.tile_pool(name="lntp", bufs=2) as lntp,
                tc.tile_pool(name="nxp", bufs=2) as nxp,
                tc.tile_pool(name="abp", bufs=2) as abp,
                tc.tile_pool(name="smallp", bufs=6) as smallp,
                tc.tile_pool(name="ps_stats", bufs=4, space="PSUM") as ps_stats,
                tc.tile_pool(name="ps_kq", bufs=2, space="PSUM") as ps_kq,
                tc.tile_pool(name="ps_v", bufs=1, space="PSUM") as ps_v,
            ):
                wq_sb = wproj.tile([P, FC, H], FP8, tag="wq")
                nc.scalar.dma_start(wq_sb[:], wq_d.ap())
                wk_sb = wproj.tile([P, FC, H], FP8, tag="wk")
                nc.scalar.dma_start(wk_sb[:], wk_d.ap())
                wv_sb = wproj.tile([P, FC, H], FP8, tag="wv")
                nc.scalar.dma_start(wv_sb[:], wv_d.ap())
                nc.scalar.dma_start(wo_sb[:], wo_d.ap())
                nc.scalar.dma_start(resid1[:], latq_ap)

                def emit_stats(tt):
                    latbf_t = latp.tile([P, FC, TQ], BF16, tag="latbf",
                                        name=f"latbf{tt}")
                    nc.sync.dma_start(latbf_t[:], latbf_ap[:, :, ts(tt, TQ)])
                    sq_t = sqp.tile([P, FC, TQ], BF16, tag="sq",
                                    name=f"sq{tt}")
                    nc.scalar.activation(sq_t[:], latbf_t[:], AF.Square)
                    ps_stat = ps_stats.tile([33, TQ], F32, tag="stats",
                                            name=f"stat{tt}")
                    for c in range(FC):
                        nc.tensor.matmul(ps_stat[0:1, :], ones_col_bf[:],
                                         latbf_t[:, c, :],
                                         start=(c == 0), stop=(c == FC - 1))
                    for c in range(FC):
                        nc.tensor.matmul(ps_stat[32:33, :], ones_col_bf[:],
                                         sq_t[:, c, :],
                                         start=(c == 0), stop=(c == FC - 1))
                    return latbf_t, ps_stat

                pend = [emit_stats(t) for t in range(NTT)]
                for tt in range(NTT):
                    latbf_t, ps_stat = pend[tt]
                    ab, bb = ln_tail(TQ, ps_stat[0:1, :], ps_stat[32:33, :],
                                     smallp, abp)
                    nx_t = nxp.tile([P, FC, TQ], FP8, tag="nx")
                    # fused whole-tile LN apply (stride-0 chunk broadcast)
                    t = lntp.tile([P, FC, TQ], BF16, tag="lnt",
                                  name=f"lnt{tt}")
                    nc.vector.tensor_mul(
                        t[:], latbf_t[:],
                        ab[:].unsqueeze(1).broadcast_to((P, FC, TQ)))
                    nc.vector.tensor_add(
                        nx_t[:], t[:],
                        bb[:].unsqueeze(1).broadcast_to((P, FC, TQ)))

                    # K projection (feature-major out)
                    for mc in range(FC):
                        ps = ps_kq.tile([P, TQ], F32, tag="kq")
                        for cp in range(FCP):
                            nc.tensor.matmul(ps[:],
                                             wk_sb[:, 2 * cp:2 * cp + 2, ts(mc, P)],
                                             nx_t[:, 2 * cp:2 * cp + 2, :],
                                             start=(cp == 0), stop=(cp == FCP - 1),
                                             perf_mode=DR)
                        nc.scalar.activation(kT[tt][:, mc, :], ps[:],
                                             AF.Identity, bias=bk_sb[:, mc:mc + 1],
                                             scale=RWS)
                    # V projection (token-major out, ones col preset); two
                    # bank-aligned 384-wide psum groups, single evacuation
                    for tcl in range(TQ // P):
                        tcg = tt * (TQ // P) + tcl
                        ps = ps_v.tile([P, 2, TQ], F32, tag="v")
                        for half in range(2):
                            for cp in range(FCP):
                                nc.tensor.matmul(
                                    ps[:, half, 0:384],
                                    nx_t[:, 2 * cp:2 * cp + 2, ts(tcl, P)],
                                    wv_sb[:, 2 * cp:2 * cp + 2,
                                          ds(half * 384, 384)],
                                    start=(cp == 0), stop=(cp == FCP - 1),
                                    perf_mode=DR)
                        nc.vector.tensor_scalar_mul(
                            v_sb[:, tcg, :, 0:DH].rearrange(
                                "p (two h) d -> p two h d", two=2),
                            ps[:, :, 0:384].rearrange(
                                "p two (h d) -> p two h d", d=DH),
                            RWS)
                    # Q projection (own tokens live in tt==0)
                    if tt == 0:
                        for mc in range(FC):
                            ps = ps_kq.tile([P, TQ], F32, tag="kq")
                            for cp in range(FCP):
                                nc.tensor.matmul(
                                    ps[:],
                                    wq_sb[:, 2 * cp:2 * cp + 2, ts(mc, P)],
                                    nx_t[:, 2 * cp:2 * cp + 2, :],
                                    start=(cp == 0), stop=(cp == FCP - 1),
                                    perf_mode=DR)
                            nc.scalar.activation(qT[:, mc, :], ps[:],
                                                 AF.Identity,
                                                 bias=bq_sb[:, mc:mc + 1],
                                                 scale=RWS)

            # ------------- Phase 2+3: attention, Wo+LN2, FFN -------------
            with (
                tc.tile_pool(name="wffn", bufs=1) as wffn,
                tc.tile_pool(name="attnp", bufs=4) as attnp,
                tc.tile_pool(name="w1sp", bufs=6) as w1sp,
                tc.tile_pool(name="rrow", bufs=2) as rrow,
                tc.tile_pool(name="rbp", bufs=1) as rbp,
                tc.tile_pool(name="sq2p", bufs=1) as sq2p,
                tc.tile_pool(name="ab2p", bufs=1) as ab2p,
                tc.tile_pool(name="small2p", bufs=5) as small2p,
                tc.tile_pool(name="hp_pool", bufs=2) as hp_pool,
            ):
                w2_sb = wffn.tile([P, FFCP, 2, H], FP8, tag="w2")
                nc.scalar.dma_start(w2_sb[:], w2_d.ap())
                with (
                    tc.tile_pool(name="ps_sc", bufs=2, space="PSUM") as ps_sc,
                    tc.tile_pool(name="ps_ctx", bufs=1, space="PSUM") as ps_ctx,
                ):
                    for hps in [(0, 1), (2, 3), (4, 5)]:
                        ctx_tiles = {}
                        for hp in hps:
                            ctxA_ps = ps_ctx.tile([DH + 1, TQ], F32,
                                                  tag=f"ctxA{hp % 2}",
                                                  name=f"ctxA{hp}")
                            ctxB_ps = ps_ctx.tile([DH + 1, TQ], F32,
                                                  tag=f"ctxB{hp % 2}",
                                                  name=f"ctxB{hp}")
                            ctx_tiles[hp] = (ctxA_ps, ctxB_ps)
                        for j in range(TKC):
                            jt, jj = j // (TQ // P), j % (TQ // P)
                            for hp in hps:
                                hA, hB = 2 * hp, 2 * hp + 1
                                sc = ps_sc.tile([P, 2, TQ], F32, tag="sc")
                                nc.tensor.matmul(sc[:, 0, :],
                                                 kT[jt][0:DH, hp, ts(jj, P)],
                                                 qT[0:DH, hp, :],
                                                 start=True, stop=True)
                                nc.tensor.matmul(sc[:, 1, :],
                                                 kT[jt][DH:P, hp, ts(jj, P)],
                                                 qT[DH:P, hp, :],
                                                 start=True, stop=True)
                                a2 = attnp.tile([P, 2, TQ], BF16, tag="attn")
                                nc.scalar.activation(a2[:], sc[:], AF.Exp,
                                                     scale=0.125,
                                                     bias=zero_col[:])
                                ctxA_ps, ctxB_ps = ctx_tiles[hp]
                                nc.tensor.matmul(ctxA_ps[:], v_sb[:, j, hA, :],
                                                 a2[:, 0, :],
                                                 start=(j == 0),
                                                 stop=(j == TKC - 1))
                                nc.tensor.matmul(ctxB_ps[:], v_sb[:, j, hB, :],
                                                 a2[:, 1, :],
                                                 start=(j == 0),
                                                 stop=(j == TKC - 1))
                        for hp in hps:
                            ctxA_ps, ctxB_ps = ctx_tiles[hp]
                            # ACT copy shifts the denom row to partition 0
                            # (custom DVE ops require offset-0 operands)
                            dA = rrow.tile([1, TQ], F32, tag="dr")
                            nc.scalar.copy(dA[:], ctxA_ps[DH:DH + 1, :])
                            dB = rrow.tile([1, TQ], F32, tag="dr")
                            nc.scalar.copy(dB[:], ctxB_ps[DH:DH + 1, :])
                            rA = rrow.tile([1, TQ], F32, tag="rr")
                            nc.vector.reciprocal_approx_fast(rA[:], dA[:])
                            rB = rrow.tile([1, TQ], F32, tag="rr")
                            nc.vector.reciprocal_approx_fast(rB[:], dB[:])
                            rb = rbp.tile([DH, 2, TQ], F32, tag="rb")
                            nc.gpsimd.partition_broadcast(rb[:, 0, :], rA[0:1, :])
                            nc.gpsimd.partition_broadcast(rb[:, 1, :], rB[0:1, :])
                            # x16 lifts ctx out of e4m3's denormal range
                            # (undone in the Wo evacuation scale)
                            nc.vector.scalar_tensor_tensor(
                                ctxP[hp][:, 0, :], ctxA_ps[0:DH, :], 16.0,
                                rb[:, 0, :], AluOpType.mult, AluOpType.mult)
                            nc.vector.scalar_tensor_tensor(
                                ctxP[hp][:, 1, :], ctxB_ps[0:DH, :], 16.0,
                                rb[:, 1, :], AluOpType.mult, AluOpType.mult)

                    # ---- Wo (DoubleRow over head pairs) + residual + LN2 ----
                    latbf2 = sq2p.tile([P, FC, TQ], BF16, tag="latbf2")
                    wo_ps = []
                    wo_tags = ["ctxA0", "ctxB0", "ctxA1", "ctxB1", "sc", "sc"]
                    for mc in range(FC):
                        pool = ps_ctx if mc < 4 else ps_sc
                        ps = pool.tile([P, TQ], F32, tag=wo_tags[mc],
                                       name=f"wops{mc}")
                        wo_ps.append(ps[:])
                    for hp in range(HPAIRS):
                        for mc in range(FC):
                            nc.tensor.matmul(wo_ps[mc],
                                             wo_sb[:, hp, :, ts(mc, P)],
                                             ctxP[hp][:],
                                             start=(hp == 0),
                                             stop=(hp == HPAIRS - 1),
                                             perf_mode=DR)
                    ps_sum2 = ps_ctx.tile([1, TQ], F32, tag="ctxA0")
                    ps_sq2 = ps_ctx.tile([33, TQ], F32, tag="ctxB0")
                    for mc in range(FC):
                        nc.vector.affine_then_add(lat2T[:, mc, :], wo_ps[mc],
                                                  resid1[:, mc, :], RWS / 16.0,
                                                  bo_sb[:, mc:mc + 1])
                        nc.scalar.copy(latbf2[:, mc, :], lat2T[:, mc, :])
                        sq2 = sq2p.tile([P, TQ], BF16, tag="sq2",
                                        name=f"sq2_{mc}")
                        nc.vector.tensor_mul(sq2[:], lat2T[:, mc, :],
                                             lat2T[:, mc, :])
                        nc.tensor.matmul(ps_sum2[0:1, :], ones_col_bf[:],
                                         latbf2[:, mc, :],
                                         start=(mc == 0), stop=(mc == FC - 1))
                        nc.tensor.matmul(ps_sq2[32:33, :], ones_col_bf[:],
                                         sq2[:],
                                         start=(mc == 0), stop=(mc == FC - 1))
                    ab2, bb2 = ln_tail(TQ, ps_sum2[0:1, :], ps_sq2[32:33, :],
                                       small2p, ab2p)
                    t2 = sq2p.tile([P, FC, TQ], BF16, tag="lnt2",
                                   name="lnt2")
                    nc.vector.tensor_mul(
                        t2[:], latbf2[:],
                        ab2[:].unsqueeze(1).broadcast_to((P, FC, TQ)))
                    nc.vector.tensor_add(
                        nx2[:], t2[:],
                        bb2[:].unsqueeze(1).broadcast_to((P, FC, TQ)))

                if DEBUG_TAPS:
                    nc.sync.dma_start(dbg_lat2_d.ap(), lat2T[:])
                    nc.sync.dma_start(dbg_nx2_d.ap(), nx2[:])
                    nc.sync.dma_start(dbg_ctx_d.ap(), ctxP[0][:])
                    nc.sync.dma_start(dbg_k_d.ap(), kT[0][:])

                # ---- FFN ----
                outT = persist.tile([P, FC, TQ], F32, tag="bigf32")
                with (
                    tc.tile_pool(name="ps_fo", bufs=1, space="PSUM") as ps_fo,
                    tc.tile_pool(name="ps_h", bufs=2, space="PSUM") as ps_h,
                ):
                    ps_out = ps_fo.tile([P, FC, TQ], F32, tag="fo")
                    for mhp in range(FFCP):
                        h2 = hp_pool.tile([P, 2, TQ], FP8, tag="h2")
                        for i in range(2):
                            mh = 2 * mhp + i
                            w1t = w1sp.tile([P, FC, P], BF16, tag="w1s",
                                            name=f"w1t{mh}")
                            nc.sync.dma_start(w1t[:], w1_d.ap()[mh])
                            # one [P,TQ] psum (= a full bank) per mh: two
                            # accumulation groups may not share a 2KB bank
                            psh = ps_h.tile([P, TQ], F32, tag="h",
                                            name=f"psh{mh}")
                            for kc in range(FC):
                                nc.tensor.matmul(psh[:],
                                                 w1t[:, kc, :],
                                                 nx2[:, kc, :],
                                                 start=(kc == 0),
                                                 stop=(kc == FC - 1))
                            nc.scalar.activation(h2[:, i, :], psh[:],
                                                 AF.Gelu,
                                                 bias=b1_sb[:, mh:mh + 1])
                        for mc in range(FC):
                            nc.tensor.matmul(ps_out[:, mc, :],
                                             w2_sb[:, mhp, :, ts(mc, P)],
                                             h2[:],
                                             start=(mhp == 0),
                                             stop=(mhp == FFCP - 1),
                                             perf_mode=DR)
                    for mc in range(FC):
                        nc.vector.affine_then_add(outT[:, mc, :],
                                                  ps_out[:, mc, :],
                                                  lat2T[:, mc, :], RWS,
                                                  b2_sb[:, mc:mc + 1])
                        nc.sync.dma_start(out_ap[:, mc, :], outT[:, mc, :])

    nc.compile()
    return nc


_NC_CACHE = {}


def _get_nc():
    if "nc" not in _NC_CACHE:
        _NC_CACHE["nc"] = build()
    return _NC_CACHE["nc"]


def _to_fp8(w):
    return (np.asarray(w, np.float32) * WS).astype(ml_dtypes.float8_e4m3)


def _split_fp8(w):
    hi = _to_fp8(w)
    lo = _to_fp8(np.asarray(w, np.float32)
                 - np.asarray(hi, np.float32) / WS)
    return hi, lo


def _prep_inputs(latent, ln1_w, ln1_b, Wq, bq, Wk, bk, Wv, bv, Wo, bo,
                 ln2_w, ln2_b, W1, b1, W2, b2):
    f32 = np.float32
    bf16 = ml_dtypes.bfloat16
    lat = np.asarray(latent, f32)
    ln1_w = np.asarray(ln1_w, f32); ln1_b = np.asarray(ln1_b, f32)
    ln2_w = np.asarray(ln2_w, f32); ln2_b = np.asarray(ln2_b, f32)
    Wq = np.asarray(Wq, f32); Wk = np.asarray(Wk, f32); Wv = np.asarray(Wv, f32)
    Wo = np.asarray(Wo, f32); W1 = np.asarray(W1, f32); W2 = np.asarray(W2, f32)
    bq = np.asarray(bq, f32); bk = np.asarray(bk, f32); bv = np.asarray(bv, f32)
    bo = np.asarray(bo, f32); b1 = np.asarray(b1, f32); b2 = np.asarray(b2, f32)

    wq_eff = ln1_w[:, None] * Wq
    wk_eff = ln1_w[:, None] * Wk
    wv_eff = ln1_w[:, None] * Wv
    bq_eff = ln1_b @ Wq + bq
    bk_eff = ln1_b @ Wk + bk
    bv_eff = ln1_b @ Wv + bv
    bo_eff = bv_eff @ Wo + bo
    w1_eff = ln2_w[:, None] * W1
    b1_eff = ln2_b @ W1 + b1

    def chunk_in(w):  # [H, M] -> [P, FC, M]  (contraction chunked)
        return np.ascontiguousarray(
            w.reshape(FC, P, -1).transpose(1, 0, 2))

    wq8 = _to_fp8(chunk_in(wq_eff))
    wk8 = _to_fp8(chunk_in(wk_eff))
    wv8 = _to_fp8(chunk_in(wv_eff))
    # Wo: [H, H] -> [DH, HPAIRS, 2, H]  rows hp*128 + i*64 + p
    wo8 = _to_fp8(np.ascontiguousarray(
        Wo.reshape(HPAIRS, 2, DH, H).transpose(2, 0, 1, 3)))
    # W1: [H, FF] -> [FFC, P, FC, P] bf16 (mh-major for streamed tiles)
    w1b = np.ascontiguousarray(
        chunk_in(w1_eff).reshape(P, FC, FFC, P).transpose(2, 0, 1, 3)
    ).astype(ml_dtypes.bfloat16)
    # W2: [FF, H] -> [P, FFCP, 2, H]  rows (2j+i)*128 + p, fp8 x64
    w28 = _to_fp8(np.ascontiguousarray(
        W2.reshape(FFCP, 2, P, H).transpose(2, 0, 1, 3)))

    def chunked(b):  # [H or FF] -> [P, nchunks]
        return np.ascontiguousarray(b.reshape(-1, P).T)

    common = {
        "wq": wq8, "wk": wk8, "wv": wv8, "wo": wo8,
        "w1": w1b, "w2": w28,
        "bq": chunked(bq_eff), "bk": chunked(bk_eff), "bo": chunked(bo_eff),
        "b1": chunked(b1_eff), "b2": chunked(b2),
    }
    in_maps = []
    for c in range(NCORES):
        b = c // (NCORES // B)
        q = c % (NCORES // B)
        latT_c = np.ascontiguousarray(np.roll(lat[b].T, -q * TQ, axis=1))
        m = dict(common)
        m["latTq"] = np.ascontiguousarray(latT_c[:, :TQ])
        m["latTbf"] = latT_c.astype(bf16)
        in_maps.append(m)
    return in_maps


def kernel(**inputs):
    nc = _get_nc()
    in_maps = _prep_inputs(**inputs)
    res = run_bass_kernel_spmd(nc, in_maps, core_ids=list(range(NCORES)))
    out = np.empty((B, S, H), np.float32)
    for c in range(NCORES):
        b = c // (NCORES // B)
        q = c % (NCORES // B)
        out[b, q * TQ:(q + 1) * TQ, :] = res.results[c]["outT"].T
    return out


# revision 3
# speedup vs baseline: 1.0154x; 1.0154x over previous
"""Trainium2 Bass kernel for a BasicTransformerBlock (B=2, S=2048, H=768, FF=3072, NH=12).

Sharding: core c handles batch b=c//4, sequence quarter q=c%4 (512 tokens).
Each core redundantly computes LN1 + K/V projections for its batch's full
2048 tokens (no collectives); Q/attention/Wo/FFN only for its own 512 tokens.

v2 over the bf16 baseline:
- fp8(e4m3, x64 pre-scale) DoubleRow matmuls for QKV projections, Wo and the
  FFN W2 GEMM (2 contraction chunks per instruction -> ~2x the bf16 rate).
  W1 stays bf16 (the z=nx@W1 path dominates the quantization error budget;
  this config lands ~1.3e-2 rel L2 vs the 2e-2 gate).
- Wo consumes per-head-pair ctx tiles [64, 2, T] via DoubleRow, which sums the
  two heads' contributions and removes the partition-shift DMAs the old
  attention tail needed.  Softmax denominators: ACT copy shifts the psum row
  to partition 0 (custom DVE ops require offset-0 operands), DVE fast-approx
  reciprocal, gpsimd partition_broadcast, and a x16 pre-scale on the fp8 ctx
  store to stay out of e4m3's denormal range (undone in the Wo evacuation).
- LN1 stats (ones-row matmuls on x and ACT-squared x) for all 4 token tiles
  are emitted up front; the LN apply is 2 fused whole-tile DVE ops using
  stride-0 chunk-broadcast APs of alpha/beta.
- Phase-1 ACT functions kept to one table set (Identity/Square/Sqrt/Copy) to
  avoid mid-phase ACT table reloads; exp is the only attention ACT function.
- W2/Wo/W1-tiles prefetched on the scalar DMA ring in need-order; W1 streams
  per-mh on the sync ring during the FFN.

Host-side folds (f32): Wq_eff = diag(ln1_w) Wq, bq_eff = ln1_b@Wq + bq (same
k/v); bo_eff = (ln1_b@Wv + bv)@Wo + bo; W1_eff = diag(ln2_w) W1,
b1_eff = ln2_b@W1 + b1.  fp8 weights are scaled by 64 before the e4m3 cast
(undone at PSUM evacuation) so weight magnitudes sit in e4m3's normal range.
"""

import os
import numpy as np
import ml_dtypes

DEBUG_TAPS = bool(int(os.environ.get("KDBG", "0")))

import concourse.bass as bass
import concourse.tile as tile
from concourse import bacc, mybir
from concourse.bass import ts, ds
from concourse.alu_op_type import AluOpType
from concourse.bass_utils import run_bass_kernel_spmd

F32 = mybir.dt.float32
BF16 = mybir.dt.bfloat16
FP8 = mybir.dt.float8e4
AF = mybir.ActivationFunctionType
DR = mybir.MatmulPerfMode.DoubleRow

H = 768
FF = 3072
NH = 12
DH = 64
B = 2
S = 2048
P = 128
NCORES = 8
TQ = 512          # own tokens per core
NTT = S // TQ     # 4 token tiles per batch
FC = H // P       # 6 feature chunks
FCP = FC // 2     # 3 feature chunk pairs
FFC = FF // P     # 24 hidden chunks
FFCP = FFC // 2   # 12 hidden chunk pairs
TKC = S // P      # 16 key token chunks
HPAIRS = NH // 2  # 6 head pairs
TH = TQ // 2      # FFN token half
EPS = 1e-6
WS = 64.0         # fp8 weight pre-scale
RWS = 1.0 / WS


def build():
    nc = bacc.Bacc("TRN2", target_bir_lowering=False, debug=False,
                   num_devices=NCORES)

    latq_d = nc.dram_tensor("latTq", [H, TQ], F32, kind="ExternalInput")
    latbf_d = nc.dram_tensor("latTbf", [H, S], BF16, kind="ExternalInput")
    wq_d = nc.dram_tensor("wq", [P, FC, H], FP8, kind="ExternalInput")
    wk_d = nc.dram_tensor("wk", [P, FC, H], FP8, kind="ExternalInput")
    wv_d = nc.dram_tensor("wv", [P, FC, H], FP8, kind="ExternalInput")
    wo_d = nc.dram_tensor("wo", [DH, HPAIRS, 2, H], FP8, kind="ExternalInput")
    w1_d = nc.dram_tensor("w1", [FFC, P, FC, P], BF16, kind="ExternalInput")
    w2_d = nc.dram_tensor("w2", [P, FFCP, 2, H], FP8, kind="ExternalInput")
    bq_d = nc.dram_tensor("bq", [P, FC], F32, kind="ExternalInput")
    bk_d = nc.dram_tensor("bk", [P, FC], F32, kind="ExternalInput")
    bo_d = nc.dram_tensor("bo", [P, FC], F32, kind="ExternalInput")
    b1_d = nc.dram_tensor("b1", [P, FFC], F32, kind="ExternalInput")
    b2_d = nc.dram_tensor("b2", [P, FC], F32, kind="ExternalInput")
    out_d = nc.dram_tensor("outT", [H, TQ], F32, kind="ExternalOutput")
    if DEBUG_TAPS:
        dbg_lat2_d = nc.dram_tensor("dbg_lat2", [P, FC, TQ], F32,
                                    kind="ExternalOutput")
        dbg_nx2_d = nc.dram_tensor("dbg_nx2", [P, FC, TQ], BF16,
                                   kind="ExternalOutput")
        dbg_ctx_d = nc.dram_tensor("dbg_ctx", [DH, 2, TQ], FP8,
                                   kind="ExternalOutput")
        dbg_k_d = nc.dram_tensor("dbg_k", [P, FC, TQ], BF16,
                                 kind="ExternalOutput")

    latq_ap = latq_d.ap().rearrange("(c p) t -> p c t", p=P)
    latbf_ap = latbf_d.ap().rearrange("(c p) t -> p c t", p=P)
    out_ap = out_d.ap().rearrange("(c p) t -> p c t", p=P)

    with tile.TileContext(nc) as tc:
        with (
            tc.tile_pool(name="consts", bufs=1) as consts,
            tc.tile_pool(name="persist", bufs=1) as persist,
        ):
            # constants (vector ring for the small bias DMAs)
            ones_col_bf = consts.tile([P, 1], BF16)
            nc.vector.memset(ones_col_bf[:], 1.0)
            eps_tile = consts.tile([1, 1], F32)
            nc.vector.memset(eps_tile[:], EPS)
            zero_col = consts.tile([P, 1], F32)
            nc.vector.memset(zero_col[:], 0.0)
            bq_sb = consts.tile([P, FC], F32)
            nc.gpsimd.dma_start(bq_sb[:], bq_d.ap())
            bk_sb = consts.tile([P, FC], F32)
            nc.gpsimd.dma_start(bk_sb[:], bk_d.ap())
            bo_sb = consts.tile([P, FC], F32)
            nc.gpsimd.dma_start(bo_sb[:], bo_d.ap())
            b1_sb = consts.tile([P, FFC], F32)
            nc.gpsimd.dma_start(b1_sb[:], b1_d.ap())
            b2_sb = consts.tile([P, FC], F32)
            nc.gpsimd.dma_start(b2_sb[:], b2_d.ap())

            # persistent activations
            kT = [persist.tile([P, FC, TQ], BF16, tag=f"kT{t}",
                               name=f"kT{t}")
                  for t in range(NTT)]
            v_sb = persist.tile([P, TKC, NH, DH + 1], BF16)
            nc.vector.memset(v_sb[:, :, :, DH:DH + 1], 1.0)
            qT = persist.tile([P, FC, TQ], BF16)
            ctxP = [persist.tile([DH, 2, TQ], FP8, tag=f"ctxP{hp}",
                                 name=f"ctxP{hp}")
                    for hp in range(HPAIRS)]
            resid1 = persist.tile([P, FC, TQ], F32, tag="bigf32")
            lat2T = persist.tile([P, FC, TQ], F32, tag="lat2")
            nx2 = persist.tile([P, FC, TQ], BF16, tag="nx2")

            wo_sb = persist.tile([DH, HPAIRS, 2, H], FP8, tag="wo")

            def ln_tail(T, ps_sum, ps_sq, small_pool, ab_pool):
                """sum/sqsum rows -> broadcast alpha/beta [P,T] bf16 tiles."""
                mu = small_pool.tile([1, T], F32, tag="lnsmall")
                nc.scalar.mul(mu[:], ps_sum, 1.0 / H)
                mu2 = small_pool.tile([1, T], F32, tag="lnsmall")
                nc.vector.tensor_mul(mu2[:], mu[:], mu[:])
                msq = small_pool.tile([1, T], F32, tag="lnsmall")
                nc.scalar.mul(msq[:], ps_sq, 1.0 / H)
                var = small_pool.tile([1, T], F32, tag="lnsmall")
                nc.vector.tensor_sub(var[:], msq[:], mu2[:])
                sd = small_pool.tile([1, T], F32, tag="lnsmall")
                nc.scalar.activation(sd[:], var[:], AF.Sqrt, bias=eps_tile[:])
                rsig = small_pool.tile([1, T], F32, tag="lnsmall")
                nc.vector.reciprocal_approx_fast(rsig[:], sd[:])
                rsig_bf = small_pool.tile([1, T], BF16, tag="lnsmallbf")
                nc.scalar.copy(rsig_bf[:], rsig[:])
                beta_bf = small_pool.tile([1, T], BF16, tag="lnsmallbf")
                nc.vector.scalar_tensor_tensor(beta_bf[:], mu[:], -1.0, rsig[:],
                                               AluOpType.mult, AluOpType.mult)
                ab = ab_pool.tile([P, T], BF16, tag="ab")
                nc.gpsimd.partition_broadcast(ab[:], rsig_bf[0:1, :])
                bb = ab_pool.tile([P, T], BF16, tag="bb")
                nc.gpsimd.partition_broadcast(bb[:], beta_bf[0:1, :])
                return ab, bb

            # ---------------- Phase 1: LN1 + K/V/Q projections ----------------
            with (
                tc.tile_pool(name="wproj", bufs=1) as wproj,
                tc.tile_pool(name="latp", bufs=4) as latp,
                tc.tile_pool(name="sqp", bufs=2) as sqp,
                tc.tile_pool(name="lntp", bufs=2) as lntp,
                tc.tile_pool(name="nxp", bufs=2) as nxp,
                tc.tile_pool(name="abp", bufs=2) as abp,
                tc.tile_pool(name="smallp", bufs=6) as smallp,
                tc.tile_pool(name="ps_stats", bufs=4, space="PSUM") as ps_stats,
                tc.tile_pool(name="ps_kq", bufs=2, space="PSUM") as ps_kq,
                tc.tile_pool(name="ps_v", bufs=1, space="PSUM") as ps_v,
            ):
                wq_sb = wproj.tile([P, FC, H], FP8, tag="wq")
                nc.scalar.dma_start(wq_sb[:], wq_d.ap())
                wk_sb = wproj.tile([P, FC, H], FP8, tag="wk")
                nc.scalar.dma_start(wk_sb[:], wk_d.ap())
                wv_sb = wproj.tile([P, FC, H], FP8, tag="wv")
                nc.scalar.dma_start(wv_sb[:], wv_d.ap())
                nc.scalar.dma_start(wo_sb[:], wo_d.ap())
                nc.scalar.dma_start(resid1[:], latq_ap)

                def emit_stats(tt):
                    latbf_t = latp.tile([P, FC, TQ], BF16, tag="latbf",
                                        name=f"latbf{tt}")
                    nc.sync.dma_start(latbf_t[:], latbf_ap[:, :, ts(tt, TQ)])
                    sq_t = sqp.tile([P, FC, TQ], BF16, tag="sq",
                                    name=f"sq{tt}")
                    nc.scalar.activation(sq_t[:], latbf_t[:], AF.Square)
                    ps_stat = ps_stats.tile([33, TQ], F32, tag="stats",
                                            name=f"stat{tt}")
                    for c in range(FC):
                        nc.tensor.matmul(ps_stat[0:1, :], ones_col_bf[:],
                                         latbf_t[:, c, :],
                                         start=(c == 0), stop=(c == FC - 1))
                    for c in range(FC):
                        nc.tensor.matmul(ps_stat[32:33, :], ones_col_bf[:],
                                         sq_t[:, c, :],
                                         start=(c == 0), stop=(c == FC - 1))
                    return latbf_t, ps_stat

                pend = [emit_stats(t) for t in range(NTT)]
                for tt in range(NTT):
                    latbf_t, ps_stat = pend[tt]
                    ab, bb = ln_tail(TQ, ps_stat[0:1, :], ps_stat[32:33, :],
                                     smallp, abp)
                    nx_t = nxp.tile([P, FC, TQ], FP8, tag="nx")
                    # fused whole-tile LN apply (stride-0 chunk broadcast)
                    t = lntp.tile([P, FC, TQ], BF16, tag="lnt",
                                  name=f"lnt{tt}")
                    nc.vector.tensor_mul(
                        t[:], latbf_t[:],
                        ab[:].unsqueeze(1).broadcast_to((P, FC, TQ)))
                    nc.vector.tensor_add(
                        nx_t[:], t[:],
                        bb[:].unsqueeze(1).broadcast_to((P, FC, TQ)))

                    # K projection (feature-major out)
                    for mc in range(FC):
                        ps = ps_kq.tile([P, TQ], F32, tag="kq")
                        for cp in range(FCP):
                            nc.tensor.matmul(ps[:],
                                             wk_sb[:, 2 * cp:2 * cp + 2, ts(mc, P)],
                                             nx_t[:, 2 * cp:2 * cp + 2, :],
                                             start=(cp == 0), stop=(cp == FCP - 1),
                                             perf_mode=DR)
                        nc.scalar.activation(kT[tt][:, mc, :], ps[:],
                                             AF.Identity, bias=bk_sb[:, mc:mc + 1],
                                             scale=RWS)
                    # V projection (token-major out, ones col preset); two
                    # bank-aligned 384-wide psum groups, single evacuation
                    for tcl in range(TQ // P):
                        tcg = tt * (TQ // P) + tcl
                        ps = ps_v.tile([P, 2, TQ], F32, tag="v")
                        for half in range(2):
                            for cp in range(FCP):
                                nc.tensor.matmul(
                                    ps[:, half, 0:384],
                                    nx_t[:, 2 * cp:2 * cp + 2, ts(tcl, P)],
                                    wv_sb[:, 2 * cp:2 * cp + 2,
                                          ds(half * 384, 384)],
                                    start=(cp == 0), stop=(cp == FCP - 1),
                                    perf_mode=DR)
                        nc.vector.tensor_scalar_mul(
                            v_sb[:, tcg, :, 0:DH].rearrange(
                                "p (two h) d -> p two h d", two=2),
                            ps[:, :, 0:384].rearrange(
                                "p two (h d) -> p two h d", d=DH),
                            RWS)
                    # Q projection (own tokens live in tt==0)
                    if tt == 0:
                        for mc in range(FC):
                            ps = ps_kq.tile([P, TQ], F32, tag="kq")
                            for cp in range(FCP):
                                nc.tensor.matmul(
                                    ps[:],
                                    wq_sb[:, 2 * cp:2 * cp + 2, ts(mc, P)],
                                    nx_t[:, 2 * cp:2 * cp + 2, :],
                                    start=(cp == 0), stop=(cp == FCP - 1),
                                    perf_mode=DR)
                            nc.scalar.activation(qT[:, mc, :], ps[:],
                                                 AF.Identity,
                                                 bias=bq_sb[:, mc:mc + 1],
                                                 scale=RWS)

            # ------------- Phase 2+3: attention, Wo+LN2, FFN -------------
            with (
                tc.tile_pool(name="wffn", bufs=1) as wffn,
                tc.tile_pool(name="attnp", bufs=6) as attnp,
                tc.tile_pool(name="w1sp", bufs=6) as w1sp,
                tc.tile_pool(name="rrow", bufs=2) as rrow,
                tc.tile_pool(name="rbp", bufs=1) as rbp,
                tc.tile_pool(name="sq2p", bufs=1) as sq2p,
                tc.tile_pool(name="ab2p", bufs=1) as ab2p,
                tc.tile_pool(name="small2p", bufs=5) as small2p,
                tc.tile_pool(name="hp_pool", bufs=2) as hp_pool,
            ):
                w2_sb = wffn.tile([P, FFCP, 2, H], FP8, tag="w2")
                nc.scalar.dma_start(w2_sb[:], w2_d.ap())
                with (
                    tc.tile_pool(name="ps_sc", bufs=2, space="PSUM") as ps_sc,
                    tc.tile_pool(name="ps_ctx", bufs=1, space="PSUM") as ps_ctx,
                ):
                    for hps in [(0, 1), (2, 3), (4, 5)]:
                        ctx_tiles = {}
                        for hp in hps:
                            ctxA_ps = ps_ctx.tile([DH + 1, TQ], F32,
                                                  tag=f"ctxA{hp % 2}",
                                                  name=f"ctxA{hp}")
                            ctxB_ps = ps_ctx.tile([DH + 1, TQ], F32,
                                                  tag=f"ctxB{hp % 2}",
                                                  name=f"ctxB{hp}")
                            ctx_tiles[hp] = (ctxA_ps, ctxB_ps)
                        for j in range(TKC):
                            jt, jj = j // (TQ // P), j % (TQ // P)
                            for hp in hps:
                                hA, hB = 2 * hp, 2 * hp + 1
                                sc = ps_sc.tile([P, 2, TQ], F32, tag="sc")
                                nc.tensor.matmul(sc[:, 0, :],
                                                 kT[jt][0:DH, hp, ts(jj, P)],
                                                 qT[0:DH, hp, :],
                                                 start=True, stop=True)
                                nc.tensor.matmul(sc[:, 1, :],
                                                 kT[jt][DH:P, hp, ts(jj, P)],
                                                 qT[DH:P, hp, :],
                                                 start=True, stop=True)
                                a2 = attnp.tile([P, 2, TQ], FP8, tag="attn")
                                nc.scalar.activation(a2[:], sc[:], AF.Exp,
                                                     scale=0.125,
                                                     bias=zero_col[:])
                                ctxA_ps, ctxB_ps = ctx_tiles[hp]
                                nc.tensor.matmul(ctxA_ps[:], v_sb[:, j, hA, :],
                                                 a2[:, 0, :],
                                                 start=(j == 0),
                                                 stop=(j == TKC - 1))
                                nc.tensor.matmul(ctxB_ps[:], v_sb[:, j, hB, :],
                                                 a2[:, 1, :],
                                                 start=(j == 0),
                                                 stop=(j == TKC - 1))
                        for hp in hps:
                            ctxA_ps, ctxB_ps = ctx_tiles[hp]
                            # ACT copy shifts the denom row to partition 0
                            # (custom DVE ops require offset-0 operands)
                            dA = rrow.tile([1, TQ], F32, tag="dr")
                            nc.scalar.copy(dA[:], ctxA_ps[DH:DH + 1, :])
                            dB = rrow.tile([1, TQ], F32, tag="dr")
                            nc.scalar.copy(dB[:], ctxB_ps[DH:DH + 1, :])
                            rA = rrow.tile([1, TQ], F32, tag="rr")
                            nc.vector.reciprocal_approx_fast(rA[:], dA[:])
                            rB = rrow.tile([1, TQ], F32, tag="rr")
                            nc.vector.reciprocal_approx_fast(rB[:], dB[:])
                            rb = rbp.tile([DH, 2, TQ], F32, tag="rb")
                            nc.gpsimd.partition_broadcast(rb[:, 0, :], rA[0:1, :])
                            nc.gpsimd.partition_broadcast(rb[:, 1, :], rB[0:1, :])
                            # x16 lifts ctx out of e4m3's denormal range
                            # (undone in the Wo evacuation scale)
                            nc.vector.scalar_tensor_tensor(
                                ctxP[hp][:, 0, :], ctxA_ps[0:DH, :], 16.0,
                                rb[:, 0, :], AluOpType.mult, AluOpType.mult)
                            nc.vector.scalar_tensor_tensor(
                                ctxP[hp][:, 1, :], ctxB_ps[0:DH, :], 16.0,
                                rb[:, 1, :], AluOpType.mult, AluOpType.mult)

                    # ---- Wo (DoubleRow over head pairs) + residual + LN2 ----
                    latbf2 = sq2p.tile([P, FC, TQ], BF16, tag="latbf2")
                    wo_ps = []
                    wo_tags = ["ctxA0", "ctxB0", "ctxA1", "ctxB1", "sc", "sc"]
                    for mc in range(FC):
                        pool = ps_ctx if mc < 4 else ps_sc
                        ps = pool.tile([P, TQ], F32, tag=wo_tags[mc],
                                       name=f"wops{mc}")
                        wo_ps.append(ps[:])
                    for hp in range(HPAIRS):
                        for mc in range(FC):
                            nc.tensor.matmul(wo_ps[mc],
                                             wo_sb[:, hp, :, ts(mc, P)],
                                             ctxP[hp][:],
                                             start=(hp == 0),
                                             stop=(hp == HPAIRS - 1),
                                             perf_mode=DR)
                    ps_sum2 = ps_ctx.tile([1, TQ], F32, tag="ctxA0")
                    ps_sq2 = ps_ctx.tile([33, TQ], F32, tag="ctxB0")
                    for mc in range(FC):
                        nc.vector.affine_then_add(lat2T[:, mc, :], wo_ps[mc],
                                                  resid1[:, mc, :], RWS / 16.0,
                                                  bo_sb[:, mc:mc + 1])
                        nc.scalar.copy(latbf2[:, mc, :], lat2T[:, mc, :])
                        sq2 = sq2p.tile([P, TQ], BF16, tag="sq2",
                                        name=f"sq2_{mc}")
                        nc.vector.tensor_mul(sq2[:], lat2T[:, mc, :],
                                             lat2T[:, mc, :])
                        nc.tensor.matmul(ps_sum2[0:1, :], ones_col_bf[:],
                                         latbf2[:, mc, :],
                                         start=(mc == 0), stop=(mc == FC - 1))
                        nc.tensor.matmul(ps_sq2[32:33, :], ones_col_bf[:],
                                         sq2[:],
                                         start=(mc == 0), stop=(mc == FC - 1))
                    ab2, bb2 = ln_tail(TQ, ps_sum2[0:1, :], ps_sq2[32:33, :],
                                       small2p, ab2p)
                    t2 = sq2p.tile([P, FC, TQ], BF16, tag="lnt2",
                                   name="lnt2")
                    nc.vector.tensor_mul(
                        t2[:], latbf2[:],
                        ab2[:].unsqueeze(1).broadcast_to((P, FC, TQ)))
                    nc.vector.tensor_add(
                        nx2[:], t2[:],
                        bb2[:].unsqueeze(1).broadcast_to((P, FC, TQ)))

                if DEBUG_TAPS:
                    nc.sync.dma_start(dbg_lat2_d.ap(), lat2T[:])
                    nc.sync.dma_start(dbg_nx2_d.ap(), nx2[:])
                    nc.sync.dma_start(dbg_ctx_d.ap(), ctxP[0][:])
                    nc.sync.dma_start(dbg_k_d.ap(), kT[0][:])

                # ---- FFN ----
                outT = persist.tile([P, FC, TQ], F32, tag="bigf32")
                with (
                    tc.tile_pool(name="ps_fo", bufs=1, space="PSUM") as ps_fo,
                    tc.tile_pool(name="ps_h", bufs=2, space="PSUM") as ps_h,
                ):
                    ps_out = ps_fo.tile([P, FC, TQ], F32, tag="fo")
                    for mhp in range(FFCP):
                        h2 = hp_pool.tile([P, 2, TQ], FP8, tag="h2")
                        for i in range(2):
                            mh = 2 * mhp + i
                            w1t = w1sp.tile([P, FC, P], BF16, tag="w1s",
                                            name=f"w1t{mh}")
                            nc.sync.dma_start(w1t[:], w1_d.ap()[mh])
                            # one [P,TQ] psum (= a full bank) per mh: two
                            # accumulation groups may not share a 2KB bank
                            psh = ps_h.tile([P, TQ], F32, tag="h",
                                            name=f"psh{mh}")
                            for kc in range(FC):
                                nc.tensor.matmul(psh[:],
                                                 w1t[:, kc, :],
                                                 nx2[:, kc, :],
                                                 start=(kc == 0),
                                                 stop=(kc == FC - 1))
                            nc.scalar.activation(h2[:, i, :], psh[:],
                                                 AF.Gelu,
                                                 bias=b1_sb[:, mh:mh + 1])
                        for mc in range(FC):
                            nc.tensor.matmul(ps_out[:, mc, :],
                                             w2_sb[:, mhp, :, ts(mc, P)],
                                             h2[:],
                                             start=(mhp == 0),
                                             stop=(mhp == FFCP - 1),
                                             perf_mode=DR)
                    for mc in range(FC):
                        nc.vector.affine_then_add(outT[:, mc, :],
                                                  ps_out[:, mc, :],
                                                  lat2T[:, mc, :], RWS,
                                                  b2_sb[:, mc:mc + 1])
                        nc.sync.dma_start(out_ap[:, mc, :], outT[:, mc, :])

    nc.compile()
    return nc


_NC_CACHE = {}


def _get_nc():
    if "nc" not in _NC_CACHE:
        _NC_CACHE["nc"] = build()
    return _NC_CACHE["nc"]


def _to_fp8(w):
    return (np.asarray(w, np.float32) * WS).astype(ml_dtypes.float8_e4m3)


def _split_fp8(w):
    hi = _to_fp8(w)
    lo = _to_fp8(np.asarray(w, np.float32)
                 - np.asarray(hi, np.float32) / WS)
    return hi, lo


def _prep_inputs(latent, ln1_w, ln1_b, Wq, bq, Wk, bk, Wv, bv, Wo, bo,
                 ln2_w, ln2_b, W1, b1, W2, b2):
    f32 = np.float32
    bf16 = ml_dtypes.bfloat16
    lat = np.asarray(latent, f32)
    ln1_w = np.asarray(ln1_w, f32); ln1_b = np.asarray(ln1_b, f32)
    ln2_w = np.asarray(ln2_w, f32); ln2_b = np.asarray(ln2_b, f32)
    Wq = np.asarray(Wq, f32); Wk = np.asarray(Wk, f32); Wv = np.asarray(Wv, f32)
    Wo = np.asarray(Wo, f32); W1 = np.asarray(W1, f32); W2 = np.asarray(W2, f32)
    bq = np.asarray(bq, f32); bk = np.asarray(bk, f32); bv = np.asarray(bv, f32)
    bo = np.asarray(bo, f32); b1 = np.asarray(b1, f32); b2 = np.asarray(b2, f32)

    wq_eff = ln1_w[:, None] * Wq
    wk_eff = ln1_w[:, None] * Wk
    wv_eff = ln1_w[:, None] * Wv
    bq_eff = ln1_b @ Wq + bq
    bk_eff = ln1_b @ Wk + bk
    bv_eff = ln1_b @ Wv + bv
    bo_eff = bv_eff @ Wo + bo
    w1_eff = ln2_w[:, None] * W1
    b1_eff = ln2_b @ W1 + b1

    def chunk_in(w):  # [H, M] -> [P, FC, M]  (contraction chunked)
        return np.ascontiguousarray(
            w.reshape(FC, P, -1).transpose(1, 0, 2))

    wq8 = _to_fp8(chunk_in(wq_eff))
    wk8 = _to_fp8(chunk_in(wk_eff))
    wv8 = _to_fp8(chunk_in(wv_eff))
    # Wo: [H, H] -> [DH, HPAIRS, 2, H]  rows hp*128 + i*64 + p
    wo8 = _to_fp8(np.ascontiguousarray(
        Wo.reshape(HPAIRS, 2, DH, H).transpose(2, 0, 1, 3)))
    # W1: [H, FF] -> [FFC, P, FC, P] bf16 (mh-major for streamed tiles)
    w1b = np.ascontiguousarray(
        chunk_in(w1_eff).reshape(P, FC, FFC, P).transpose(2, 0, 1, 3)
    ).astype(ml_dtypes.bfloat16)
    # W2: [FF, H] -> [P, FFCP, 2, H]  rows (2j+i)*128 + p, fp8 x64
    w28 = _to_fp8(np.ascontiguousarray(
        W2.reshape(FFCP, 2, P, H).transpose(2, 0, 1, 3)))

    def chunked(b):  # [H or FF] -> [P, nchunks]
        return np.ascontiguousarray(b.reshape(-1, P).T)

    common = {
        "wq": wq8, "wk": wk8, "wv": wv8, "wo": wo8,
        "w1": w1b, "w2": w28,
        "bq": chunked(bq_eff), "bk": chunked(bk_eff), "bo": chunked(bo_eff),
        "b1": chunked(b1_eff), "b2": chunked(b2),
    }
    in_maps = []
    for c in range(NCORES):
        b = c // (NCORES // B)
        q = c % (NCORES // B)
        latT_c = np.ascontiguousarray(np.roll(lat[b].T, -q * TQ, axis=1))
        m = dict(common)
        m["latTq"] = np.ascontiguousarray(latT_c[:, :TQ])
        m["latTbf"] = latT_c.astype(bf16)
        in_maps.append(m)
    return in_maps


def kernel(**inputs):
    nc = _get_nc()
    in_maps = _prep_inputs(**inputs)
    res = run_bass_kernel_spmd(nc, in_maps, core_ids=list(range(NCORES)))
    out = np.empty((B, S, H), np.float32)
    for c in range(NCORES):
        b = c // (NCORES // B)
        q = c % (NCORES // B)
        out[b, q * TQ:(q + 1) * TQ, :] = res.results[c]["outT"].T
    return out


# revision 5
# speedup vs baseline: 1.0271x; 1.0115x over previous
"""Trainium2 Bass kernel for a BasicTransformerBlock (B=2, S=2048, H=768, FF=3072, NH=12).

Sharding: core c handles batch b=c//4, sequence quarter q=c%4 (512 tokens).
Each core redundantly computes LN1 + K/V projections for its batch's full
2048 tokens (no collectives); Q/attention/Wo/FFN only for its own 512 tokens.

v2 over the bf16 baseline:
- fp8(e4m3, x64 pre-scale) DoubleRow matmuls for QKV projections, Wo and the
  FFN W2 GEMM (2 contraction chunks per instruction -> ~2x the bf16 rate).
  W1 stays bf16 (the z=nx@W1 path dominates the quantization error budget;
  this config lands ~1.3e-2 rel L2 vs the 2e-2 gate).
- Wo consumes per-head-pair ctx tiles [64, 2, T] via DoubleRow, which sums the
  two heads' contributions and removes the partition-shift DMAs the old
  attention tail needed.  Softmax denominators: ACT copy shifts the psum row
  to partition 0 (custom DVE ops require offset-0 operands), DVE fast-approx
  reciprocal, gpsimd partition_broadcast, and a x16 pre-scale on the fp8 ctx
  store to stay out of e4m3's denormal range (undone in the Wo evacuation).
- LN1 stats (ones-row matmuls on x and ACT-squared x) for all 4 token tiles
  are emitted up front; the LN apply is 2 fused whole-tile DVE ops using
  stride-0 chunk-broadcast APs of alpha/beta.
- Phase-1 ACT functions kept to one table set (Identity/Square/Sqrt/Copy) to
  avoid mid-phase ACT table reloads; exp is the only attention ACT function.
- W2/Wo/W1-tiles prefetched on the scalar DMA ring in need-order; W1 streams
  per-mh on the sync ring during the FFN.

Host-side folds (f32): Wq_eff = diag(ln1_w) Wq, bq_eff = ln1_b@Wq + bq (same
k/v); bo_eff = (ln1_b@Wv + bv)@Wo + bo; W1_eff = diag(ln2_w) W1,
b1_eff = ln2_b@W1 + b1.  fp8 weights are scaled by 64 before the e4m3 cast
(undone at PSUM evacuation) so weight magnitudes sit in e4m3's normal range.
"""

import os
import numpy as np
import ml_dtypes

DEBUG_TAPS = bool(int(os.environ.get("KDBG", "0")))

import concourse.bass as bass
import concourse.tile as tile
from concourse import bacc, mybir
from concourse.bass import ts, ds
from concourse.alu_op_type import AluOpType
from concourse.bass_utils import run_bass_kernel_spmd

F32 = mybir.dt.float32
BF16 = mybir.dt.bfloat16
FP8 = mybir.dt.float8e4
AF = mybir.ActivationFunctionType
DR = mybir.MatmulPerfMode.DoubleRow

H = 768
FF = 3072
NH = 12
DH = 64
B = 2
S = 2048
P = 128
NCORES = 8
TQ = 512          # own tokens per core
NTT = S // TQ     # 4 token tiles per batch
FC = H // P       # 6 feature chunks
FCP = FC // 2     # 3 feature chunk pairs
FFC = FF // P     # 24 hidden chunks
FFCP = FFC // 2   # 12 hidden chunk pairs
TKC = S // P      # 16 key token chunks
HPAIRS = NH // 2  # 6 head pairs
TH = TQ // 2      # FFN token half
EPS = 1e-6
WS = 64.0         # fp8 weight pre-scale
RWS = 1.0 / WS


def build():
    nc = bacc.Bacc("TRN2", target_bir_lowering=False, debug=False,
                   num_devices=NCORES)

    latq_d = nc.dram_tensor("latTq", [H, TQ], F32, kind="ExternalInput")
    latbf_d = nc.dram_tensor("latTbf", [H, S], BF16, kind="ExternalInput")
    wq_d = nc.dram_tensor("wq", [P, FC, H], FP8, kind="ExternalInput")
    wk_d = nc.dram_tensor("wk", [P, FC, H], FP8, kind="ExternalInput")
    wv_d = nc.dram_tensor("wv", [P, FC, H], FP8, kind="ExternalInput")
    wo_d = nc.dram_tensor("wo", [DH, HPAIRS, 2, H], FP8, kind="ExternalInput")
    w1_d = nc.dram_tensor("w1", [FFC, P, FC, P], BF16, kind="ExternalInput")
    w2_d = nc.dram_tensor("w2", [P, FFCP, 2, H], FP8, kind="ExternalInput")
    bq_d = nc.dram_tensor("bq", [P, FC], F32, kind="ExternalInput")
    bk_d = nc.dram_tensor("bk", [P, FC], F32, kind="ExternalInput")
    bo_d = nc.dram_tensor("bo", [P, FC], F32, kind="ExternalInput")
    b1_d = nc.dram_tensor("b1", [P, FFC], F32, kind="ExternalInput")
    b2_d = nc.dram_tensor("b2", [P, FC], F32, kind="ExternalInput")
    out_d = nc.dram_tensor("outT", [H, TQ], F32, kind="ExternalOutput")
    if DEBUG_TAPS:
        dbg_lat2_d = nc.dram_tensor("dbg_lat2", [P, FC, TQ], F32,
                                    kind="ExternalOutput")
        dbg_nx2_d = nc.dram_tensor("dbg_nx2", [P, FC, TQ], BF16,
                                   kind="ExternalOutput")
        dbg_ctx_d = nc.dram_tensor("dbg_ctx", [DH, 2, TQ], FP8,
                                   kind="ExternalOutput")
        dbg_k_d = nc.dram_tensor("dbg_k", [P, FC, TQ], BF16,
                                 kind="ExternalOutput")

    latq_ap = latq_d.ap().rearrange("(c p) t -> p c t", p=P)
    latbf_ap = latbf_d.ap().rearrange("(c p) t -> p c t", p=P)
    out_ap = out_d.ap().rearrange("(c p) t -> p c t", p=P)

    with tile.TileContext(nc) as tc:
        with (
            tc.tile_pool(name="consts", bufs=1) as consts,
            tc.tile_pool(name="persist", bufs=1) as persist,
        ):
            # constants (vector ring for the small bias DMAs)
            ones_col_bf = consts.tile([P, 1], BF16)
            nc.vector.memset(ones_col_bf[:], 1.0)
            eps_tile = consts.tile([1, 1], F32)
            nc.vector.memset(eps_tile[:], EPS)
            zero_col = consts.tile([P, 1], F32)
            nc.vector.memset(zero_col[:], 0.0)
            bq_sb = consts.tile([P, FC], F32)
            nc.gpsimd.dma_start(bq_sb[:], bq_d.ap())
            bk_sb = consts.tile([P, FC], F32)
            nc.gpsimd.dma_start(bk_sb[:], bk_d.ap())
            bo_sb = consts.tile([P, FC], F32)
            nc.gpsimd.dma_start(bo_sb[:], bo_d.ap())
            b1_sb = consts.tile([P, FFC], F32)
            nc.gpsimd.dma_start(b1_sb[:], b1_d.ap())
            b2_sb = consts.tile([P, FC], F32)
            nc.gpsimd.dma_start(b2_sb[:], b2_d.ap())

            # persistent activations
            kT = [persist.tile([P, FC, TQ], BF16, tag=f"kT{t}",
                               name=f"kT{t}")
                  for t in range(NTT)]
            v_sb = persist.tile([P, TKC, NH, DH + 1], BF16)
            nc.vector.memset(v_sb[:, :, :, DH:DH + 1], 1.0)
            qT = persist.tile([P, FC, TQ], BF16)
            ctxP = [persist.tile([DH, 2, TQ], FP8, tag=f"ctxP{hp}",
                                 name=f"ctxP{hp}")
                    for hp in range(HPAIRS)]
            resid1 = persist.tile([P, FC, TQ], F32, tag="bigf32")
            lat2T = persist.tile([P, FC, TQ], F32, tag="lat2")
            nx2 = persist.tile([P, FC, TQ], BF16, tag="nx2")

            wo_sb = persist.tile([DH, HPAIRS, 2, H], FP8, tag="wo")

            def ln_tail(T, ps_sum, ps_sq, small_pool, ab_pool):
                """sum/sqsum rows -> broadcast alpha/beta [P,T] bf16 tiles."""
                mu = small_pool.tile([1, T], F32, tag="lnsmall")
                nc.scalar.mul(mu[:], ps_sum, 1.0 / H)
                mu2 = small_pool.tile([1, T], F32, tag="lnsmall")
                nc.vector.tensor_mul(mu2[:], mu[:], mu[:])
                msq = small_pool.tile([1, T], F32, tag="lnsmall")
                nc.scalar.mul(msq[:], ps_sq, 1.0 / H)
                var = small_pool.tile([1, T], F32, tag="lnsmall")
                nc.vector.tensor_sub(var[:], msq[:], mu2[:])
                sd = small_pool.tile([1, T], F32, tag="lnsmall")
                nc.scalar.activation(sd[:], var[:], AF.Sqrt, bias=eps_tile[:])
                rsig = small_pool.tile([1, T], F32, tag="lnsmall")
                nc.vector.reciprocal_approx_fast(rsig[:], sd[:])
                rsig_bf = small_pool.tile([1, T], BF16, tag="lnsmallbf")
                nc.scalar.copy(rsig_bf[:], rsig[:])
                beta_bf = small_pool.tile([1, T], BF16, tag="lnsmallbf")
                nc.vector.scalar_tensor_tensor(beta_bf[:], mu[:], -1.0, rsig[:],
                                               AluOpType.mult, AluOpType.mult)
                ab = ab_pool.tile([P, T], BF16, tag="ab")
                nc.gpsimd.partition_broadcast(ab[:], rsig_bf[0:1, :])
                bb = ab_pool.tile([P, T], BF16, tag="bb")
                nc.gpsimd.partition_broadcast(bb[:], beta_bf[0:1, :])
                return ab, bb

            # ---------------- Phase 1: LN1 + K/V/Q projections ----------------
            with (
                tc.tile_pool(name="wproj", bufs=1) as wproj,
                tc.tile_pool(name="latp", bufs=4) as latp,
                tc.tile_pool(name="sqp", bufs=2) as sqp,
                tc.tile_pool(name="lntp", bufs=2) as lntp,
                tc.tile_pool(name="nxp", bufs=2) as nxp,
                tc.tile_pool(name="abp", bufs=2) as abp,
                tc.tile_pool(name="smallp", bufs=6) as smallp,
                tc.tile_pool(name="ps_stats", bufs=4, space="PSUM") as ps_stats,
                tc.tile_pool(name="ps_kq", bufs=2, space="PSUM") as ps_kq,
                tc.tile_pool(name="ps_v", bufs=1, space="PSUM") as ps_v,
            ):
                wq_sb = wproj.tile([P, FC, H], FP8, tag="wq")
                nc.scalar.dma_start(wq_sb[:], wq_d.ap())
                wk_sb = wproj.tile([P, FC, H], FP8, tag="wk")
                nc.scalar.dma_start(wk_sb[:], wk_d.ap())
                wv_sb = wproj.tile([P, FC, H], FP8, tag="wv")
                nc.scalar.dma_start(wv_sb[:], wv_d.ap())
                nc.scalar.dma_start(wo_sb[:], wo_d.ap())
                nc.scalar.dma_start(resid1[:], latq_ap)

                def emit_stats(tt):
                    latbf_t = latp.tile([P, FC, TQ], BF16, tag="latbf",
                                        name=f"latbf{tt}")
                    nc.sync.dma_start(latbf_t[:], latbf_ap[:, :, ts(tt, TQ)])
                    sq_t = sqp.tile([P, FC, TQ], BF16, tag="sq",
                                    name=f"sq{tt}")
                    nc.scalar.activation(sq_t[:], latbf_t[:], AF.Square)
                    ps_stat = ps_stats.tile([33, TQ], F32, tag="stats",
                                            name=f"stat{tt}")
                    for c in range(FC):
                        nc.tensor.matmul(ps_stat[0:1, :], ones_col_bf[:],
                                         latbf_t[:, c, :],
                                         start=(c == 0), stop=(c == FC - 1))
                    for c in range(FC):
                        nc.tensor.matmul(ps_stat[32:33, :], ones_col_bf[:],
                                         sq_t[:, c, :],
                                         start=(c == 0), stop=(c == FC - 1))
                    return latbf_t, ps_stat

                pend = [emit_stats(t) for t in range(NTT)]
                for tt in range(NTT):
                    latbf_t, ps_stat = pend[tt]
                    ab, bb = ln_tail(TQ, ps_stat[0:1, :], ps_stat[32:33, :],
                                     smallp, abp)
                    nx_t = nxp.tile([P, FC, TQ], FP8, tag="nx")
                    # fused whole-tile LN apply (stride-0 chunk broadcast)
                    t = lntp.tile([P, FC, TQ], BF16, tag="lnt",
                                  name=f"lnt{tt}")
                    nc.vector.tensor_mul(
                        t[:], latbf_t[:],
                        ab[:].unsqueeze(1).broadcast_to((P, FC, TQ)))
                    nc.vector.tensor_add(
                        nx_t[:], t[:],
                        bb[:].unsqueeze(1).broadcast_to((P, FC, TQ)))

                    # K projection (feature-major out)
                    for mc in range(FC):
                        ps = ps_kq.tile([P, TQ], F32, tag="kq")
                        for cp in range(FCP):
                            nc.tensor.matmul(ps[:],
                                             wk_sb[:, 2 * cp:2 * cp + 2, ts(mc, P)],
                                             nx_t[:, 2 * cp:2 * cp + 2, :],
                                             start=(cp == 0), stop=(cp == FCP - 1),
                                             perf_mode=DR)
                        nc.scalar.activation(kT[tt][:, mc, :], ps[:],
                                             AF.Identity, bias=bk_sb[:, mc:mc + 1],
                                             scale=RWS)
                    # V projection (token-major out, ones col preset); two
                    # bank-aligned 384-wide psum groups, single evacuation
                    for tcl in range(TQ // P):
                        tcg = tt * (TQ // P) + tcl
                        ps = ps_v.tile([P, 2, TQ], F32, tag="v")
                        for half in range(2):
                            for cp in range(FCP):
                                nc.tensor.matmul(
                                    ps[:, half, 0:384],
                                    nx_t[:, 2 * cp:2 * cp + 2, ts(tcl, P)],
                                    wv_sb[:, 2 * cp:2 * cp + 2,
                                          ds(half * 384, 384)],
                                    start=(cp == 0), stop=(cp == FCP - 1),
                                    perf_mode=DR)
                        nc.vector.tensor_scalar_mul(
                            v_sb[:, tcg, :, 0:DH].rearrange(
                                "p (two h) d -> p two h d", two=2),
                            ps[:, :, 0:384].rearrange(
                                "p two (h d) -> p two h d", d=DH),
                            RWS)
                    # Q projection (own tokens live in tt==0)
                    if tt == 0:
                        for mc in range(FC):
                            ps = ps_kq.tile([P, TQ], F32, tag="kq")
                            for cp in range(FCP):
                                nc.tensor.matmul(
                                    ps[:],
                                    wq_sb[:, 2 * cp:2 * cp + 2, ts(mc, P)],
                                    nx_t[:, 2 * cp:2 * cp + 2, :],
                                    start=(cp == 0), stop=(cp == FCP - 1),
                                    perf_mode=DR)
                            nc.scalar.activation(qT[:, mc, :], ps[:],
                                                 AF.Identity,
                                                 bias=bq_sb[:, mc:mc + 1],
                                                 scale=RWS)

            # ------------- Phase 2+3: attention, Wo+LN2, FFN -------------
            with (
                tc.tile_pool(name="wffn", bufs=1) as wffn,
                tc.tile_pool(name="attnp", bufs=8) as attnp,
                tc.tile_pool(name="w1sp", bufs=6) as w1sp,
                tc.tile_pool(name="rrow", bufs=2) as rrow,
                tc.tile_pool(name="rbp", bufs=2) as rbp,
                tc.tile_pool(name="sq2p", bufs=1) as sq2p,
                tc.tile_pool(name="ab2p", bufs=1) as ab2p,
                tc.tile_pool(name="small2p", bufs=5) as small2p,
                tc.tile_pool(name="hp_pool", bufs=2) as hp_pool,
            ):
                w2_sb = wffn.tile([P, FFCP, 2, H], FP8, tag="w2")
                nc.scalar.dma_start(w2_sb[:], w2_d.ap())
                with (
                    tc.tile_pool(name="ps_sc", bufs=2, space="PSUM") as ps_sc,
                    tc.tile_pool(name="ps_ctx", bufs=1, space="PSUM") as ps_ctx,
                ):
                    for hps in [(0, 1), (2, 3), (4, 5)]:
                        ctx_tiles = {}
                        for hp in hps:
                            ctxA_ps = ps_ctx.tile([DH + 1, TQ], F32,
                                                  tag=f"ctxA{hp % 2}",
                                                  name=f"ctxA{hp}")
                            ctxB_ps = ps_ctx.tile([DH + 1, TQ], F32,
                                                  tag=f"ctxB{hp % 2}",
                                                  name=f"ctxB{hp}")
                            ctx_tiles[hp] = (ctxA_ps, ctxB_ps)
                        for j in range(TKC):
                            jt, jj = j // (TQ // P), j % (TQ // P)
                            for hp in hps:
                                hA, hB = 2 * hp, 2 * hp + 1
                                sc = ps_sc.tile([P, 2, TQ], F32, tag="sc")
                                nc.tensor.matmul(sc[:, 0, :],
                                                 kT[jt][0:DH, hp, ts(jj, P)],
                                                 qT[0:DH, hp, :],
                                                 start=True, stop=True)
                                nc.tensor.matmul(sc[:, 1, :],
                                                 kT[jt][DH:P, hp, ts(jj, P)],
                                                 qT[DH:P, hp, :],
                                                 start=True, stop=True)
                                a2 = attnp.tile([P, 2, TQ], FP8, tag="attn")
                                nc.scalar.activation(a2[:], sc[:], AF.Exp,
                                                     scale=0.125,
                                                     bias=zero_col[:])
                                ctxA_ps, ctxB_ps = ctx_tiles[hp]
                                nc.tensor.matmul(ctxA_ps[:], v_sb[:, j, hA, :],
                                                 a2[:, 0, :],
                                                 start=(j == 0),
                                                 stop=(j == TKC - 1))
                                nc.tensor.matmul(ctxB_ps[:], v_sb[:, j, hB, :],
                                                 a2[:, 1, :],
                                                 start=(j == 0),
                                                 stop=(j == TKC - 1))
                        for hp in hps:
                            ctxA_ps, ctxB_ps = ctx_tiles[hp]
                            # ACT copy shifts the denom row to partition 0
                            # (custom DVE ops require offset-0 operands)
                            dA = rrow.tile([1, TQ], F32, tag="dr")
                            nc.vector.tensor_scalar_mul(
                                dA[:], ctxA_ps[DH:DH + 1, :], 1.0)
                            dB = rrow.tile([1, TQ], F32, tag="dr")
                            nc.vector.tensor_scalar_mul(
                                dB[:], ctxB_ps[DH:DH + 1, :], 1.0)
                            rA = rrow.tile([1, TQ], F32, tag="rr")
                            nc.vector.reciprocal_approx_fast(rA[:], dA[:])
                            rB = rrow.tile([1, TQ], F32, tag="rr")
                            nc.vector.reciprocal_approx_fast(rB[:], dB[:])
                            rb = rbp.tile([DH, 2, TQ], F32, tag="rb")
                            nc.gpsimd.partition_broadcast(rb[:, 0, :], rA[0:1, :])
                            nc.gpsimd.partition_broadcast(rb[:, 1, :], rB[0:1, :])
                            # x16 lifts ctx out of e4m3's denormal range
                            # (undone in the Wo evacuation scale)
                            nc.vector.scalar_tensor_tensor(
                                ctxP[hp][:, 0, :], ctxA_ps[0:DH, :], 16.0,
                                rb[:, 0, :], AluOpType.mult, AluOpType.mult)
                            nc.vector.scalar_tensor_tensor(
                                ctxP[hp][:, 1, :], ctxB_ps[0:DH, :], 16.0,
                                rb[:, 1, :], AluOpType.mult, AluOpType.mult)

                    # ---- Wo (DoubleRow over head pairs) + residual + LN2 ----
                    latbf2 = sq2p.tile([P, FC, TQ], BF16, tag="latbf2")
                    wo_ps = []
                    wo_tags = ["ctxA0", "ctxB0", "ctxA1", "ctxB1", "sc", "sc"]
                    for mc in range(FC):
                        pool = ps_ctx if mc < 4 else ps_sc
                        ps = pool.tile([P, TQ], F32, tag=wo_tags[mc],
                                       name=f"wops{mc}")
                        wo_ps.append(ps[:])
                    for hp in range(HPAIRS):
                        for mc in range(FC):
                            nc.tensor.matmul(wo_ps[mc],
                                             wo_sb[:, hp, :, ts(mc, P)],
                                             ctxP[hp][:],
                                             start=(hp == 0),
                                             stop=(hp == HPAIRS - 1),
                                             perf_mode=DR)
                    ps_sum2 = ps_ctx.tile([1, TQ], F32, tag="ctxA0")
                    ps_sq2 = ps_ctx.tile([33, TQ], F32, tag="ctxB0")
                    for mc in range(FC):
                        nc.vector.affine_then_add(lat2T[:, mc, :], wo_ps[mc],
                                                  resid1[:, mc, :], RWS / 16.0,
                                                  bo_sb[:, mc:mc + 1])
                        nc.scalar.copy(latbf2[:, mc, :], lat2T[:, mc, :])
                        sq2 = sq2p.tile([P, TQ], BF16, tag="sq2",
                                        name=f"sq2_{mc}")
                        nc.vector.tensor_mul(sq2[:], lat2T[:, mc, :],
                                             lat2T[:, mc, :])
                        nc.tensor.matmul(ps_sum2[0:1, :], ones_col_bf[:],
                                         latbf2[:, mc, :],
                                         start=(mc == 0), stop=(mc == FC - 1))
                        nc.tensor.matmul(ps_sq2[32:33, :], ones_col_bf[:],
                                         sq2[:],
                                         start=(mc == 0), stop=(mc == FC - 1))
                    ab2, bb2 = ln_tail(TQ, ps_sum2[0:1, :], ps_sq2[32:33, :],
                                       small2p, ab2p)
                    t2 = sq2p.tile([P, FC, TQ], BF16, tag="lnt2",
                                   name="lnt2")
                    nc.vector.tensor_mul(
                        t2[:], latbf2[:],
                        ab2[:].unsqueeze(1).broadcast_to((P, FC, TQ)))
                    nc.vector.tensor_add(
                        nx2[:], t2[:],
                        bb2[:].unsqueeze(1).broadcast_to((P, FC, TQ)))

                if DEBUG_TAPS:
                    nc.sync.dma_start(dbg_lat2_d.ap(), lat2T[:])
                    nc.sync.dma_start(dbg_nx2_d.ap(), nx2[:])
                    nc.sync.dma_start(dbg_ctx_d.ap(), ctxP[0][:])
                    nc.sync.dma_start(dbg_k_d.ap(), kT[0][:])

                # ---- FFN ----
                outT = persist.tile([P, FC, TQ], F32, tag="bigf32")
                with (
                    tc.tile_pool(name="ps_fo", bufs=1, space="PSUM") as ps_fo,
                    tc.tile_pool(name="ps_h", bufs=2, space="PSUM") as ps_h,
                ):
                    ps_out = ps_fo.tile([P, FC, TQ], F32, tag="fo")
                    for mhp in range(FFCP):
                        h2 = hp_pool.tile([P, 2, TQ], FP8, tag="h2")
                        for i in range(2):
                            mh = 2 * mhp + i
                            w1t = w1sp.tile([P, FC, P], BF16, tag="w1s",
                                            name=f"w1t{mh}")
                            nc.sync.dma_start(w1t[:], w1_d.ap()[mh])
                            # one [P,TQ] psum (= a full bank) per mh: two
                            # accumulation groups may not share a 2KB bank
                            psh = ps_h.tile([P, TQ], F32, tag="h",
                                            name=f"psh{mh}")
                            for kc in range(FC):
                                nc.tensor.matmul(psh[:],
                                                 w1t[:, kc, :],
                                                 nx2[:, kc, :],
                                                 start=(kc == 0),
                                                 stop=(kc == FC - 1))
                            nc.scalar.activation(h2[:, i, :], psh[:],
                                                 AF.Gelu,
                                                 bias=b1_sb[:, mh:mh + 1])
                        for mc in range(FC):
                            nc.tensor.matmul(ps_out[:, mc, :],
                                             w2_sb[:, mhp, :, ts(mc, P)],
                                             h2[:],
                                             start=(mhp == 0),
                                             stop=(mhp == FFCP - 1),
                                             perf_mode=DR)
                    for mc in range(FC):
                        nc.vector.affine_then_add(outT[:, mc, :],
                                                  ps_out[:, mc, :],
                                                  lat2T[:, mc, :], RWS,
                                                  b2_sb[:, mc:mc + 1])
                        nc.sync.dma_start(out_ap[:, mc, :], outT[:, mc, :])

    nc.compile()
    return nc


_NC_CACHE = {}


def _get_nc():
    if "nc" not in _NC_CACHE:
        _NC_CACHE["nc"] = build()
    return _NC_CACHE["nc"]


def _to_fp8(w):
    return (np.asarray(w, np.float32) * WS).astype(ml_dtypes.float8_e4m3)


def _split_fp8(w):
    hi = _to_fp8(w)
    lo = _to_fp8(np.asarray(w, np.float32)
                 - np.asarray(hi, np.float32) / WS)
    return hi, lo


def _prep_inputs(latent, ln1_w, ln1_b, Wq, bq, Wk, bk, Wv, bv, Wo, bo,
                 ln2_w, ln2_b, W1, b1, W2, b2):
    f32 = np.float32
    bf16 = ml_dtypes.bfloat16
    lat = np.asarray(latent, f32)
    ln1_w = np.asarray(ln1_w, f32); ln1_b = np.asarray(ln1_b, f32)
    ln2_w = np.asarray(ln2_w, f32); ln2_b = np.asarray(ln2_b, f32)
    Wq = np.asarray(Wq, f32); Wk = np.asarray(Wk, f32); Wv = np.asarray(Wv, f32)
    Wo = np.asarray(Wo, f32); W1 = np.asarray(W1, f32); W2 = np.asarray(W2, f32)
    bq = np.asarray(bq, f32); bk = np.asarray(bk, f32); bv = np.asarray(bv, f32)
    bo = np.asarray(bo, f32); b1 = np.asarray(b1, f32); b2 = np.asarray(b2, f32)

    wq_eff = ln1_w[:, None] * Wq
    wk_eff = ln1_w[:, None] * Wk
    wv_eff = ln1_w[:, None] * Wv
    bq_eff = ln1_b @ Wq + bq
    bk_eff = ln1_b @ Wk + bk
    bv_eff = ln1_b @ Wv + bv
    bo_eff = bv_eff @ Wo + bo
    w1_eff = ln2_w[:, None] * W1
    b1_eff = ln2_b @ W1 + b1

    def chunk_in(w):  # [H, M] -> [P, FC, M]  (contraction chunked)
        return np.ascontiguousarray(
            w.reshape(FC, P, -1).transpose(1, 0, 2))

    wq8 = _to_fp8(chunk_in(wq_eff))
    wk8 = _to_fp8(chunk_in(wk_eff))
    wv8 = _to_fp8(chunk_in(wv_eff))
    # Wo: [H, H] -> [DH, HPAIRS, 2, H]  rows hp*128 + i*64 + p
    wo8 = _to_fp8(np.ascontiguousarray(
        Wo.reshape(HPAIRS, 2, DH, H).transpose(2, 0, 1, 3)))
    # W1: [H, FF] -> [FFC, P, FC, P] bf16 (mh-major for streamed tiles)
    w1b = np.ascontiguousarray(
        chunk_in(w1_eff).reshape(P, FC, FFC, P).transpose(2, 0, 1, 3)
    ).astype(ml_dtypes.bfloat16)
    # W2: [FF, H] -> [P, FFCP, 2, H]  rows (2j+i)*128 + p, fp8 x64
    w28 = _to_fp8(np.ascontiguousarray(
        W2.reshape(FFCP, 2, P, H).transpose(2, 0, 1, 3)))

    def chunked(b):  # [H or FF] -> [P, nchunks]
        return np.ascontiguousarray(b.reshape(-1, P).T)

    common = {
        "wq": wq8, "wk": wk8, "wv": wv8, "wo": wo8,
        "w1": w1b, "w2": w28,
        "bq": chunked(bq_eff), "bk": chunked(bk_eff), "bo": chunked(bo_eff),
        "b1": chunked(b1_eff), "b2": chunked(b2),
    }
    in_maps = []
    for c in range(NCORES):
        b = c // (NCORES // B)
        q = c % (NCORES // B)
        latT_c = np.ascontiguousarray(np.roll(lat[b].T, -q * TQ, axis=1))
        m = dict(common)
        m["latTq"] = np.ascontiguousarray(latT_c[:, :TQ])
        m["latTbf"] = latT_c.astype(bf16)
        in_maps.append(m)
    return in_maps


def kernel(**inputs):
    nc = _get_nc()
    in_maps = _prep_inputs(**inputs)
    res = run_bass_kernel_spmd(nc, in_maps, core_ids=list(range(NCORES)))
    out = np.empty((B, S, H), np.float32)
    for c in range(NCORES):
        b = c // (NCORES // B)
        q = c % (NCORES // B)
        out[b, q * TQ:(q + 1) * TQ, :] = res.results[c]["outT"].T
    return out
